# revision 2
# baseline (speedup 1.0000x reference)
"""Trainium2 Bass kernel: segment_sum of edge features into nodes (GNN aggregation).

Computes out[n, :] = sum over edges e with receivers[e] == n of edges[e, :],
for n in [0, 100000), edges [1000000, 64] fp32 — distributed over 8 NeuronCores.

Strategy:
  - Host: value-shard edges across 8 cores by receiver range (12500 nodes/core),
    sort each core's edges by receiver, and pack them into 128-edge "chunks"
    grouped by 128-node windows (fixed 12 chunks per window, padded).
    Edge fp32 values are split losslessly-enough into fp16 hi + fp16 lo halves.
  - Device (per core): for each chunk, build a one-hot [128 tokens x 128 nodes]
    matrix on the VectorEngine (is_equal vs an iota row), then one TensorEngine
    matmul per chunk accumulates hi|lo partial sums for the window into PSUM.
    A VectorEngine add folds hi+lo and the result streams out contiguously.
  - No cross-core reduction needed: node ranges are disjoint; host concatenates.
"""

import os

import numpy as np

N_EDGES = 1_000_000
N_NODES = 100_000
N_FEAT = 64
N_CORES = 8
NODES_PER_CORE = N_NODES // N_CORES  # 12500
WIN = 128
N_WIN = (NODES_PER_CORE + WIN - 1) // WIN  # 98
K_CHUNKS = 12  # chunks (of 128 edges) per 128-node window; capacity 1536 edges
C_CHUNKS = N_WIN * K_CHUNKS  # 1176
GROUP_W = 7  # windows per DMA/flush group; 98 = 14 * 7
CAP = K_CHUNKS * WIN  # per-window edge capacity

_NC_CACHE = None
LAST_RESULT = None  # BassKernelResults of the most recent hardware run


def _build_nc():
    global _NC_CACHE
    if _NC_CACHE is not None:
        return _NC_CACHE

    import concourse.tile as tile
    from concourse import bacc, mybir

    F16 = mybir.dt.float16
    F32 = mybir.dt.float32

    nc = bacc.Bacc("TRN2", target_bir_lowering=False)
    tokens = nc.dram_tensor("tokens", [128, C_CHUNKS, 128], F16, kind="ExternalInput")
    rel = nc.dram_tensor("rel", [128, C_CHUNKS], F32, kind="ExternalInput")
    iota = nc.dram_tensor("iota", [128, 128], F16, kind="ExternalInput")
    out = nc.dram_tensor("out", [128, N_WIN, 64], F32, kind="ExternalOutput")

    n_groups = N_WIN // GROUP_W
    with tile.TileContext(nc) as tc:
        with (
            tc.tile_pool(name="const", bufs=1) as const,
            tc.tile_pool(name="tok", bufs=2) as tokp,
            tc.tile_pool(name="oh", bufs=4) as ohp,
            tc.tile_pool(name="ps", bufs=2, space="PSUM") as psp,
            tc.tile_pool(name="stage", bufs=2) as stp,
        ):
            iota_t = const.tile([128, 128], F16)
            nc.sync.dma_start(iota_t[:], iota[:])
            rel_t = const.tile([128, C_CHUNKS], F32)
            nc.sync.dma_start(rel_t[:], rel[:])

            for g in range(n_groups):
                c0 = g * GROUP_W * K_CHUNKS
                tok = tokp.tile([128, GROUP_W * K_CHUNKS, 128], F16)
                nc.sync.dma_start(tok[:], tokens[:, c0 : c0 + GROUP_W * K_CHUNKS, :])
                stage = stp.tile([128, GROUP_W * 64], F32)
                for wi in range(GROUP_W):
                    w = g * GROUP_W + wi
                    ps = psp.tile([128, 64], F32)
                    for c in range(K_CHUNKS):
                        gc = w * K_CHUNKS + c
                        oh = ohp.tile([128, 128], F16)
                        nc.vector.tensor_scalar(
                            out=oh[:],
                            in0=iota_t[:],
                            scalar1=rel_t[:, gc : gc + 1],
                            scalar2=None,
                            op0=mybir.AluOpType.is_equal,
                        )
                        nc.tensor.matmul(
                            out=ps[:],
                            lhsT=oh[:],
                            rhs=tok[:, wi * K_CHUNKS + c, 0:64],
                            start=(c == 0),
                            stop=False,
                        )
                        nc.tensor.matmul(
                            out=ps[:],
                            lhsT=oh[:],
                            rhs=tok[:, wi * K_CHUNKS + c, 64:128],
                            start=False,
                            stop=(c == K_CHUNKS - 1),
                        )
                    nc.scalar.copy(stage[:, wi * 64 : (wi + 1) * 64], ps[:])
                nc.sync.dma_start(out[:, g * GROUP_W : (g + 1) * GROUP_W, :], stage[:])
    nc.compile()
    _NC_CACHE = nc
    return nc


def _numpy_segment_sum(edges, receivers):
    out = np.zeros((N_NODES, N_FEAT), np.float32)
    r = np.asarray(receivers).astype(np.int64)
    ok = (r >= 0) & (r < N_NODES)
    np.add.at(out, r[ok], np.asarray(edges, np.float32)[ok])
    return out


def kernel(edges, nodes, receivers):
    global LAST_RESULT

    edges = np.ascontiguousarray(edges, dtype=np.float32)
    n_nodes = nodes.shape[0]
    r = np.asarray(receivers).astype(np.int64)
    if (
        edges.shape != (N_EDGES, N_FEAT)
        or n_nodes != N_NODES
        or r.shape != (N_EDGES,)
        or os.environ.get("KERNEL_FORCE_NUMPY")
    ):
        return _numpy_segment_sum(edges, receivers)

    # ---- host-side sharding / packing ----
    order = np.argsort(r, kind="stable")
    r_s = r[order]
    bounds = np.searchsorted(r_s, NODES_PER_CORE * np.arange(N_CORES + 1))

    hi_all = edges.astype(np.float16)
    lo_all = (edges - hi_all.astype(np.float32)).astype(np.float16)

    iota = np.broadcast_to(np.arange(128, dtype=np.float16), (128, 128)).copy()

    in_maps = []
    spill_ids = []
    for i in range(N_CORES):
        lo_b, hi_b = bounds[i], bounds[i + 1]
        idx = order[lo_b:hi_b]
        rr = r_s[lo_b:hi_b] - NODES_PER_CORE * i
        w = rr >> 7
        cnt = np.bincount(w, minlength=N_WIN)
        start = np.zeros(N_WIN, np.int64)
        np.cumsum(cnt[:-1], out=start[1:])
        rank = np.arange(len(rr)) - start[w]
        keep = rank < CAP
        if not keep.all():
            spill_ids.append(idx[~keep])
            idx, rr, w, rank = idx[keep], rr[keep], w[keep], rank[keep]

        tokens = np.zeros((128, C_CHUNKS, 128), np.float16)
        relarr = np.full((128, C_CHUNKS), -1.0, np.float32)
        gchunk = w * K_CHUNKS + (rank >> 7)
        slot = rank & 127
        tokens[slot, gchunk, 0:64] = hi_all[idx]
        tokens[slot, gchunk, 64:128] = lo_all[idx]
        relarr[slot, gchunk] = (rr & 127).astype(np.float32)
        in_maps.append({"tokens": tokens, "rel": relarr, "iota": iota})

    # ---- device run ----
    from concourse.bass_utils import run_bass_kernel_spmd

    nc = _build_nc()
    res = run_bass_kernel_spmd(nc, in_maps, core_ids=list(range(N_CORES)))
    LAST_RESULT = res

    # ---- unshard ----
    full = np.empty((N_NODES, N_FEAT), np.float32)
    for i in range(N_CORES):
        dev = res.results[i]["out"]  # [128, N_WIN, 64]
        part = dev.transpose(1, 0, 2).reshape(N_WIN * 128, 64)[:NODES_PER_CORE]
        full[i * NODES_PER_CORE : (i + 1) * NODES_PER_CORE] = part

    if spill_ids:
        sp = np.concatenate(spill_ids)
        np.add.at(full, r[sp], edges[sp])

    return full
